# revision 34
# baseline (speedup 1.0000x reference)
"""Trainium2 Bass kernel for nn_AbstractRelu (DeepPoly abstract-ReLU transform).

The reference's piecewise-linear transform reduces exactly to:
    x_out    = relu(x)
    high_out = relu(high)        (crossing branch: w_high*high + b_high == high)
    low_out  = low if low + high >= 0 else 0
and `relu(high)` can replace `high` in the low_out test without changing any
result (when high <= 0, low < high <= 0 forces low + high < 0 AND low < 0).

The problem is pure memory bandwidth (elementwise, 6 streams); the binding
resource is the per-core SBUF AXI fabric (~435 GB/s measured). The 2e-2
rel-err budget admits bf16 for everything except the branch decision, so:
  - x is pre-cast to bf16 on the host (relu preserves sign, so only the
    bf16 rounding of the value itself shows up: rel err ~2^-8),
  - low/high are read in f32 (the mask low+high>=0 must match the f32
    reference bit-exactly -- a flipped boundary element is rel err 1.0),
  - all three outputs are written as bf16 and upcast to f32 on the host.
Per-core traffic: 4 MiB (x) + 16 MiB (low,high) reads + 12 MiB writes
= 32 MiB vs 48 MiB all-f32; floor = 32 MiB / 435 GB/s = 77 us, measured
~78-82 us (vs 134 us for the all-f32 version).

Schedule (the "v3" default): both HWDGE rings (sync/scalar) carry ONLY
loads so no compute op ever head-of-line blocks a load issue; all compute
runs on DVE; all stores (compute-dependent by nature) go through SWDGE
(gpsimd). 0.5-1 MiB transfers (tile_cols=2048), 6 pool buffers.

Sharding: N=16.7M elements split evenly across 8 NeuronCores; fully
elementwise, no communication.
"""

import numpy as np

import concourse.bass as bass
import concourse.bacc as bacc
import concourse.mybir as mybir
from concourse.tile import TileContext
from concourse.bass_utils import run_bass_kernel_spmd

N = 16777216
N_CORES = 8
SHARD = N // N_CORES  # 2_097_152
P = 128
FREE = SHARD // P  # 16384 elements per partition per core
TILE_COLS = 2048  # 1 MiB f32 / 0.5 MiB bf16 per DMA; 8 tiles per core
N_TILES = FREE // TILE_COLS
F32 = mybir.dt.float32
BF16 = mybir.dt.bfloat16
NP_BF16 = mybir.dt.np(BF16)


def build_program(
    free: int = FREE,
    tile_cols: int = TILE_COLS,
    bufs: int = 6,
    repeats: int = 1,
    hw_loop_repeats: int = 1,
    store_engine: str = "gpsimd",
    load_engine: str = "split",
    x_relu_on_dve: bool = False,
    compute: bool = True,
    schedule: str = "v3",
) -> bass.Bass:
    """hw_loop_repeats wraps the whole body in a tc.For_i hardware loop --
    used only by the timing harness (repeat-differencing)."""
    assert free % tile_cols == 0
    n_tiles = free // tile_cols

    nc = bacc.Bacc(
        "TRN2", target_bir_lowering=False, debug=False, num_devices=N_CORES
    )
    # Each DRAM tile [P, tile_cols] is one fully contiguous block in HBM
    # (best row-buffer locality); the host reshapes to match.
    shape = [n_tiles, P, tile_cols]
    if schedule.startswith("v7"):
        # One input tensor per tile: high | low | x (bf16 riding as f32
        # bit-pairs), one bf16 output tensor: high_out | low_out | x_out.
        # 2 DMAs per tile instead of 6, identical byte counts.
        assert tile_cols % 2 == 0
        in_all = nc.declare_dram_parameter(
            "in_all", [n_tiles, P, 2 * tile_cols + tile_cols // 2], F32,
            isOutput=False,
        )
        out_all = nc.declare_dram_parameter(
            "out_all", [n_tiles, P, 3 * tile_cols], BF16, isOutput=True
        )
        x = x_out = low = high = low_out = high_out = hl = holo = None
    elif schedule.startswith("v6"):
        # high and low packed side by side per tile (cols 0:C / C:2C), and
        # likewise high_out/low_out: halves the DMA count for 4 of the 6
        # streams at identical byte counts.
        x = nc.declare_dram_parameter("x", shape, BF16, isOutput=False)
        x_out = nc.declare_dram_parameter("x_out", shape, BF16, isOutput=True)
        shape2 = [n_tiles, P, 2 * tile_cols]
        hl = nc.declare_dram_parameter("hl", shape2, F32, isOutput=False)
        holo = nc.declare_dram_parameter("holo_out", shape2, BF16, isOutput=True)
        low = high = low_out = high_out = None
    else:
        x = nc.declare_dram_parameter("x", shape, BF16, isOutput=False)
        x_out = nc.declare_dram_parameter("x_out", shape, BF16, isOutput=True)
        low = nc.declare_dram_parameter("low", shape, F32, isOutput=False)
        high = nc.declare_dram_parameter("high", shape, F32, isOutput=False)
        low_out = nc.declare_dram_parameter("low_out", shape, BF16, isOutput=True)
        high_out = nc.declare_dram_parameter("high_out", shape, BF16, isOutput=True)

    relu = mybir.ActivationFunctionType.Relu

    if schedule == "v8":
        # v3 engine roles at tile_cols=4096 with SPLIT tile pools: the
        # load tiles (x,h,l: 40 KB/partition/buf) get a 4-deep pool while
        # the short-lived output tiles (ho,lo: 16 KB) need only 2 -- 192 KB
        # total. Halves the per-op DVE fixed overhead vs C=2048 (20 ops
        # instead of 40 per iteration) while keeping the same bytes of
        # load-ahead slack as bufs=8 at C=2048.
        with TileContext(nc) as tc:
            with tc.tile_pool(name="ld", bufs=bufs) as lpool:
                with tc.tile_pool(name="st", bufs=2) as spool:

                    def body_v8():
                        C = tile_cols
                        for t in range(n_tiles * repeats):
                            ti = t % n_tiles
                            ht = lpool.tile([P, C], F32, tag="h")
                            nc.scalar.dma_start(out=ht[:], in_=high[ti])
                            lt = lpool.tile([P, C], F32, tag="l")
                            nc.sync.dma_start(out=lt[:], in_=low[ti])
                            xt = lpool.tile([P, C], BF16, tag="x")
                            (nc.sync if t % 2 == 0 else nc.scalar).dma_start(
                                out=xt[:], in_=x[ti]
                            )

                            nc.vector.tensor_scalar_max(xt[:], xt[:], 0.0)
                            nc.gpsimd.dma_start(out=x_out[ti], in_=xt[:])

                            ho = spool.tile([P, C], BF16, tag="ho")
                            nc.vector.tensor_scalar_max(ho[:], ht[:], 0.0)
                            nc.gpsimd.dma_start(out=high_out[ti], in_=ho[:])

                            nc.vector.tensor_add(ht[:], lt[:], ht[:])
                            nc.vector.tensor_scalar(
                                ht[:], ht[:], 0.0, None, mybir.AluOpType.is_ge
                            )
                            lo = spool.tile([P, C], BF16, tag="lo")
                            nc.vector.tensor_mul(lo[:], ht[:], lt[:])
                            nc.gpsimd.dma_start(out=low_out[ti], in_=lo[:])

                    if hw_loop_repeats > 1:
                        with tc.For_i(0, hw_loop_repeats, 1):
                            body_v8()
                    else:
                        body_v8()
        nc.compile()
        return nc

    with TileContext(nc) as tc:
        with tc.tile_pool(name="io", bufs=bufs) as pool:
            engines = {"scalar": nc.scalar, "gpsimd": nc.gpsimd, "sync": nc.sync}

            def eng_for(stream: str, t: int):
                """Resolve the DMA-issuing engine for stream in
                {x,h,l,xo,ho,lo} at tile t. Loads stay on the two HWDGE
                rings (sync/scalar) so they are never head-of-line blocked
                behind stores, which wait on compute; stores go to SWDGE
                (gpsimd) by default."""
                if stream in ("x", "h", "l"):
                    if load_engine == "split":
                        # balance HWDGE ring bytes: h(f32) on scalar,
                        # l(f32) on sync, x(bf16, half-size) alternates
                        if stream == "h":
                            return engines["scalar"]
                        if stream == "l":
                            return engines["sync"]
                        return engines["sync" if t % 2 == 0 else "scalar"]
                    return engines[load_engine]
                if store_engine == "mix":
                    return engines["scalar" if stream == "xo" else "gpsimd"]
                if store_engine == "alt":
                    return engines["gpsimd" if t % 2 == 0 else "scalar"]
                return engines[store_engine]

            def body():
                for t in range(n_tiles * repeats):
                    ti = t % n_tiles

                    xt = pool.tile([P, tile_cols], BF16, tag="x")
                    eng_for("x", t).dma_start(out=xt[:], in_=x[ti])
                    if compute:
                        if x_relu_on_dve:
                            nc.vector.tensor_scalar_max(xt[:], xt[:], 0.0)
                        else:
                            nc.scalar.activation(xt[:], xt[:], relu)
                    eng_for("xo", t).dma_start(out=x_out[ti], in_=xt[:])

                    ht = pool.tile([P, tile_cols], F32, tag="h")
                    eng_for("h", t).dma_start(out=ht[:], in_=high[ti])
                    lt = pool.tile([P, tile_cols], F32, tag="l")
                    eng_for("l", t).dma_start(out=lt[:], in_=low[ti])

                    if not compute:
                        # DMA-floor diagnostic: identical transfer shapes,
                        # no compute ops (stores the loaded bytes as-is)
                        eng_for("ho", t).dma_start(
                            out=high_out[ti],
                            in_=ht[:].bitcast(BF16)[:, 0:tile_cols],
                        )
                        eng_for("lo", t).dma_start(
                            out=low_out[ti],
                            in_=lt[:].bitcast(BF16)[:, 0:tile_cols],
                        )
                        continue

                    ho = pool.tile([P, tile_cols], BF16, tag="ho")
                    nc.scalar.activation(ho[:], ht[:], relu)  # f32 -> bf16
                    eng_for("ho", t).dma_start(out=high_out[ti], in_=ho[:])

                    # s = low + high computed in place over ht (f32, exact);
                    # mask = (s >= 0); low_out = mask * low, rounded to bf16
                    nc.vector.tensor_add(ht[:], lt[:], ht[:])
                    nc.vector.tensor_scalar(
                        ht[:], ht[:], 0.0, None, mybir.AluOpType.is_ge
                    )
                    lo = pool.tile([P, tile_cols], BF16, tag="lo")
                    nc.vector.tensor_mul(lo[:], ht[:], lt[:])
                    eng_for("lo", t).dma_start(out=low_out[ti], in_=lo[:])

            def body_v3():
                """Both HWDGE rings are pure load streams; all compute on
                DVE; all stores on SWDGE."""
                for t in range(n_tiles * repeats):
                    ti = t % n_tiles

                    ht = pool.tile([P, tile_cols], F32, tag="h")
                    nc.scalar.dma_start(out=ht[:], in_=high[ti])
                    lt = pool.tile([P, tile_cols], F32, tag="l")
                    nc.sync.dma_start(out=lt[:], in_=low[ti])
                    xt = pool.tile([P, tile_cols], BF16, tag="x")
                    (nc.sync if t % 2 == 0 else nc.scalar).dma_start(
                        out=xt[:], in_=x[ti]
                    )

                    nc.vector.tensor_scalar_max(xt[:], xt[:], 0.0)
                    nc.gpsimd.dma_start(out=x_out[ti], in_=xt[:])

                    ho = pool.tile([P, tile_cols], BF16, tag="ho")
                    nc.vector.tensor_scalar_max(ho[:], ht[:], 0.0)  # f32->bf16
                    nc.gpsimd.dma_start(out=high_out[ti], in_=ho[:])

                    nc.vector.tensor_add(ht[:], lt[:], ht[:])
                    nc.vector.tensor_scalar(
                        ht[:], ht[:], 0.0, None, mybir.AluOpType.is_ge
                    )
                    lo = pool.tile([P, tile_cols], BF16, tag="lo")
                    nc.vector.tensor_mul(lo[:], ht[:], lt[:])
                    nc.gpsimd.dma_start(out=low_out[ti], in_=lo[:])

            def body_v4(prefetch: int = 2, ho_on_dve: bool = False):
                """Software-prefetched loads: tile t+PF's loads are issued
                before tile t's compute in every engine's program order, so
                a compute op on scalar never delays a load issue by more
                than the PF-tile slack. Loads: h->scalar, l->sync, x
                alternating; relus on scalar (DVE keeps only the 3-op low
                chain); stores on gpsimd."""
                total = n_tiles * repeats

                def issue_loads(t):
                    ti = t % n_tiles
                    ht = pool.tile([P, tile_cols], F32, tag="h")
                    nc.scalar.dma_start(out=ht[:], in_=high[ti])
                    lt = pool.tile([P, tile_cols], F32, tag="l")
                    nc.sync.dma_start(out=lt[:], in_=low[ti])
                    xt = pool.tile([P, tile_cols], BF16, tag="x")
                    (nc.sync if t % 2 == 0 else nc.scalar).dma_start(
                        out=xt[:], in_=x[ti]
                    )
                    return xt, ht, lt

                from collections import deque

                q = deque()
                for t in range(min(prefetch, total)):
                    q.append(issue_loads(t))
                for t in range(total):
                    if t + prefetch < total:
                        q.append(issue_loads(t + prefetch))
                    ti = t % n_tiles
                    xt, ht, lt = q.popleft()

                    nc.scalar.activation(xt[:], xt[:], relu)
                    nc.gpsimd.dma_start(out=x_out[ti], in_=xt[:])

                    ho = pool.tile([P, tile_cols], BF16, tag="ho")
                    if ho_on_dve == "alt":
                        if t % 2 == 0:
                            nc.vector.tensor_scalar_max(ho[:], ht[:], 0.0)
                        else:
                            nc.scalar.activation(ho[:], ht[:], relu)
                    elif ho_on_dve:
                        nc.vector.tensor_scalar_max(ho[:], ht[:], 0.0)
                    else:
                        nc.scalar.activation(ho[:], ht[:], relu)
                    nc.gpsimd.dma_start(out=high_out[ti], in_=ho[:])

                    nc.vector.tensor_add(ht[:], lt[:], ht[:])
                    nc.vector.tensor_scalar(
                        ht[:], ht[:], 0.0, None, mybir.AluOpType.is_ge
                    )
                    lo = pool.tile([P, tile_cols], BF16, tag="lo")
                    nc.vector.tensor_mul(lo[:], ht[:], lt[:])
                    nc.gpsimd.dma_start(out=low_out[ti], in_=lo[:])

            def body_v6(x_on_scalar: bool = False):
                """v3 engine roles with fused streams: one 2C-wide f32 load
                carries high|low, one 2C-wide bf16 store carries
                high_out|low_out; compute addresses the halves as slices."""
                C = tile_cols
                for t in range(n_tiles * repeats):
                    ti = t % n_tiles

                    hlt = pool.tile([P, 2 * C], F32, tag="hl")
                    (nc.sync if t % 2 == 0 else nc.scalar).dma_start(
                        out=hlt[:], in_=hl[ti]
                    )
                    xt = pool.tile([P, C], BF16, tag="x")
                    (nc.scalar if t % 2 == 0 else nc.sync).dma_start(
                        out=xt[:], in_=x[ti]
                    )

                    if x_on_scalar:
                        nc.scalar.activation(xt[:], xt[:], relu)
                    else:
                        nc.vector.tensor_scalar_max(xt[:], xt[:], 0.0)
                    nc.gpsimd.dma_start(out=x_out[ti], in_=xt[:])

                    ht = hlt[:, 0:C]
                    lt = hlt[:, C : 2 * C]
                    holot = pool.tile([P, 2 * C], BF16, tag="holo")
                    hot = holot[:, 0:C]
                    lot = holot[:, C : 2 * C]

                    nc.vector.tensor_scalar_max(hot, ht, 0.0)  # f32->bf16
                    nc.vector.tensor_add(ht, lt, ht)
                    nc.vector.tensor_scalar(
                        ht, ht, 0.0, None, mybir.AluOpType.is_ge
                    )
                    nc.vector.tensor_mul(lot, ht, lt)
                    nc.gpsimd.dma_start(out=holo[ti], in_=holot[:])

            def body_v7():
                """Maximal fusion: ONE f32 load (high|low|x-bits) and ONE
                bf16 store (high_out|low_out|x_out) per tile; loads
                alternate HWDGE rings, stores on SWDGE, compute on DVE."""
                C = tile_cols
                for t in range(n_tiles * repeats):
                    ti = t % n_tiles

                    it = pool.tile([P, 2 * C + C // 2], F32, tag="in")
                    (nc.sync if t % 2 == 0 else nc.scalar).dma_start(
                        out=it[:], in_=in_all[ti]
                    )
                    ot = pool.tile([P, 3 * C], BF16, tag="out")

                    ht = it[:, 0:C]
                    lt = it[:, C : 2 * C]
                    xt = it[:, 2 * C : 2 * C + C // 2].bitcast(BF16)

                    nc.vector.tensor_scalar_max(ot[:, 2 * C : 3 * C], xt, 0.0)
                    nc.vector.tensor_scalar_max(ot[:, 0:C], ht, 0.0)
                    nc.vector.tensor_add(ht, lt, ht)
                    nc.vector.tensor_scalar(
                        ht, ht, 0.0, None, mybir.AluOpType.is_ge
                    )
                    nc.vector.tensor_mul(ot[:, C : 2 * C], ht, lt)
                    nc.gpsimd.dma_start(out=out_all[ti], in_=ot[:])

            # Note: a DRAM->DRAM max-accum DMA ("relu in the SDMA CCE",
            # bypassing SBUF for the x stream) was tried and is rejected by
            # the compiler (assertDMACopySupportedCceOp).
            body_fn = {
                "v1": body,
                "v3": body_v3,
                "v4": body_v4,
                "v4d": lambda: body_v4(ho_on_dve=True),
                "v4e": lambda: body_v4(ho_on_dve="alt"),
                "v4dp3": lambda: body_v4(prefetch=3, ho_on_dve=True),
                "v4p3": lambda: body_v4(prefetch=3),
                "v4p4": lambda: body_v4(prefetch=4),
                "v6": body_v6,
                "v6s": lambda: body_v6(x_on_scalar=True),
                "v7": body_v7,
            }[schedule]
            if hw_loop_repeats > 1:
                with tc.For_i(0, hw_loop_repeats, 1):
                    body_fn()
            else:
                body_fn()
    nc.compile()
    return nc


def pack_full(x, low, high, tile_cols: int = TILE_COLS, schedule: str = "v3"):
    """Full f32 arrays -> dict of full arrays in the declared dtypes/layout
    (x cast to bf16; contiguous-tile layout [N_CORES*n_tiles, P, C]; fused
    schedules pack high|low side by side per tile)."""
    x = np.ascontiguousarray(np.asarray(x, dtype=np.float32).reshape(-1))
    low = np.ascontiguousarray(np.asarray(low, dtype=np.float32).reshape(-1))
    high = np.ascontiguousarray(np.asarray(high, dtype=np.float32).reshape(-1))
    assert x.shape == (N,), x.shape
    nt = N // (P * tile_cols)
    xb = x.astype(NP_BF16).reshape(nt, P, tile_cols)  # round-to-nearest-even
    if schedule.startswith("v7"):
        in_all = np.concatenate(
            [
                high.reshape(nt, P, tile_cols),
                low.reshape(nt, P, tile_cols),
                np.ascontiguousarray(xb).view(np.float32),  # bf16 bit-pairs
            ],
            axis=2,
        )
        return {"in_all": in_all}
    if schedule.startswith("v6"):
        hl = np.concatenate(
            [high.reshape(nt, P, tile_cols), low.reshape(nt, P, tile_cols)],
            axis=2,
        )
        return {"x": xb, "hl": hl}
    return {
        "x": xb,
        "low": low.reshape(nt, P, tile_cols),
        "high": high.reshape(nt, P, tile_cols),
    }


def unpack_outputs(by_name, schedule: str = "v3"):
    """name -> full bf16 array, back to (x_out, low_out, high_out) f32."""
    if schedule.startswith("v7"):
        oa = np.asarray(by_name["out_all"])
        C = oa.shape[-1] // 3
        return (
            oa[..., 2 * C : 3 * C].reshape(-1).astype(np.float32),
            oa[..., C : 2 * C].reshape(-1).astype(np.float32),
            oa[..., 0:C].reshape(-1).astype(np.float32),
        )
    x_out = np.asarray(by_name["x_out"]).reshape(-1).astype(np.float32)
    if schedule.startswith("v6"):
        holo = np.asarray(by_name["holo_out"])
        C = holo.shape[-1] // 2
        high_out = holo[..., 0:C].reshape(-1).astype(np.float32)
        low_out = holo[..., C : 2 * C].reshape(-1).astype(np.float32)
    else:
        low_out = np.asarray(by_name["low_out"]).reshape(-1).astype(np.float32)
        high_out = np.asarray(by_name["high_out"]).reshape(-1).astype(np.float32)
    return x_out, low_out, high_out


def shard_inputs(x, low, high, tile_cols: int = TILE_COLS, schedule: str = "v3"):
    """Per-core input dicts (spmd fallback / timing harness)."""
    full = pack_full(x, low, high, tile_cols, schedule)
    nt_core = FREE // tile_cols
    in_maps = []
    for c in range(N_CORES):
        s = slice(c * nt_core, (c + 1) * nt_core)
        in_maps.append({nm: a[s] for nm, a in full.items()})
    return in_maps


_NC = None

DEFAULT_SCHEDULE = "v3"

# Build configuration used by kernel(); test.py may override before the
# first kernel() call to validate a specific variant end to end.
DEFAULT_BUILD: dict = {}


def _get_nc() -> bass.Bass:
    global _NC
    if _NC is None:
        _NC = build_program(**DEFAULT_BUILD)
    return _NC


_RUNNER = None


def _make_runner(nc):
    """Cached PJRT runner (mirrors bass2jax.run_bass_via_pjrt, but the jitted
    callable is built once so repeat kernel() calls skip re-tracing). No
    donation: this kernel writes every output element, so the zero 'output'
    operands are reusable dummies and XLA result buffers may start uninit."""
    import jax
    from jax.sharding import Mesh, PartitionSpec, NamedSharding
    from jax.experimental.shard_map import shard_map
    from concourse.bass2jax import (
        _bass_exec_p,
        install_neuronx_cc_hook,
        partition_id_tensor,
    )

    install_neuronx_cc_hook()
    partition_name = nc.partition_id_tensor.name if nc.partition_id_tensor else None

    in_names, out_names, out_avals, zero_shapes = [], [], [], []
    in_shapes = {}
    for alloc in nc.m.functions[0].allocations:
        if not isinstance(alloc, mybir.MemoryLocationSet):
            continue
        name = alloc.memorylocations[0].name
        if alloc.kind == "ExternalInput":
            if name != partition_name:
                in_names.append(name)
                in_shapes[name] = tuple(alloc.tensor_shape)
        elif alloc.kind == "ExternalOutput":
            shape = tuple(alloc.tensor_shape)
            dtype = mybir.dt.np(alloc.dtype)
            out_names.append(name)
            out_avals.append(jax.core.ShapedArray(shape, dtype))
            zero_shapes.append((shape, dtype))
    n_params = len(in_names)
    all_in_names = list(in_names) + list(out_names)
    if partition_name is not None:
        all_in_names.append(partition_name)

    def _body(*args):
        operands = list(args)
        if partition_name is not None:
            operands.append(partition_id_tensor())
        outs = _bass_exec_p.bind(
            *operands,
            out_avals=tuple(out_avals),
            in_names=tuple(all_in_names),
            out_names=tuple(out_names),
            lowering_input_output_aliases=(),
            sim_require_finite=True,
            sim_require_nnan=True,
            nc=nc,
        )
        return tuple(outs)

    devices = jax.devices()[:N_CORES]
    mesh = Mesh(np.asarray(devices), ("core",))
    n_io = n_params + len(out_names)
    sharded = jax.jit(
        shard_map(
            _body,
            mesh=mesh,
            in_specs=(PartitionSpec("core"),) * n_io,
            out_specs=(PartitionSpec("core"),) * len(out_names),
            check_rep=False,
        ),
        keep_unused=True,
    )
    sharding = NamedSharding(mesh, PartitionSpec("core"))
    zeros = [
        jax.device_put(np.zeros((N_CORES * s[0], *s[1:]), d), sharding)
        for (s, d) in zero_shapes
    ]

    def run(full_by_name):
        """full_by_name: name -> full flat array already in the declared
        dtype. Sharding across cores is just the row-major split of axis 0
        after reshaping to (N_CORES*n_tiles, P, tile_cols), so device_put
        with the core sharding does the whole distribution."""
        dev_in = []
        for nm in in_names:
            s = in_shapes[nm]
            a = full_by_name[nm].reshape((N_CORES * s[0],) + tuple(s[1:]))
            dev_in.append(jax.device_put(a, sharding))
        outs = sharded(*dev_in, *zeros)
        return {nm: np.asarray(outs[i]) for i, nm in enumerate(out_names)}

    return run


def kernel(x: np.ndarray, low: np.ndarray, high: np.ndarray, **_run_kwargs):
    nc = _get_nc()
    tile_cols = DEFAULT_BUILD.get("tile_cols", TILE_COLS)
    schedule = DEFAULT_BUILD.get("schedule", DEFAULT_SCHEDULE)
    global _RUNNER
    results = None
    if not _run_kwargs:
        # Fast path: cached jitted executable (no per-call re-trace), fed
        # with full arrays (device_put's row-major axis-0 split IS the
        # per-core sharding).
        try:
            if _RUNNER is None:
                _RUNNER = _make_runner(nc)
            by_name = _RUNNER(pack_full(x, low, high, tile_cols, schedule))
            return unpack_outputs(by_name, schedule)
        except Exception:
            _RUNNER = None
            results = None

    if results is None:
        in_maps = shard_inputs(x, low, high, tile_cols, schedule)
        res = None
        for attempt in range(3):
            try:
                res = run_bass_kernel_spmd(
                    nc, in_maps, list(range(N_CORES)), **_run_kwargs
                )
                break
            except Exception:
                # Transient device wedge (NRT_EXEC_UNIT_UNRECOVERABLE) -- reset
                # the jax backend so the next attempt re-establishes the mesh.
                if attempt == 2:
                    raise
                import time as _time

                try:
                    import jax

                    jax.clear_caches()
                    jax.extend.backend.clear_backends()
                except Exception:
                    pass
                _time.sleep(10.0)
        results = res.results
        if _run_kwargs:
            kernel.last_results = res  # expose trace/profile to test harness

    by_name = {
        nm: np.concatenate([np.asarray(results[c][nm]) for c in range(N_CORES)])
        for nm in results[0]
    }
    return unpack_outputs(by_name, schedule)


# revision 36
# speedup vs baseline: 1.1301x; 1.1301x over previous
"""Trainium2 Bass kernel for nn_AbstractRelu (DeepPoly abstract-ReLU transform).

The reference's piecewise-linear transform reduces exactly to:
    x_out    = relu(x)
    high_out = relu(high)        (crossing branch: w_high*high + b_high == high)
    low_out  = low if low + high >= 0 else 0
and `relu(high)` can replace `high` in the low_out test without changing any
result (when high <= 0, low < high <= 0 forces low + high < 0 AND low < 0).

The problem is pure memory bandwidth (elementwise, 6 streams); the binding
resource is the per-core SBUF AXI fabric (~435 GB/s measured). The 2e-2
rel-err budget admits bf16 for everything except the branch decision, so:
  - x is pre-cast to bf16 on the host (relu preserves sign, so only the
    bf16 rounding of the value itself shows up: rel err ~2^-8),
  - low/high are read in f32 (the mask low+high>=0 must match the f32
    reference bit-exactly -- a flipped boundary element is rel err 1.0),
  - all three outputs are written as bf16 and upcast to f32 on the host.
Per-core traffic: 4 MiB (x) + 16 MiB (low,high) reads + 12 MiB writes
= 32 MiB vs 48 MiB all-f32; floor = 32 MiB / 435 GB/s = 77 us, measured
~78-82 us (vs 134 us for the all-f32 version).

Schedule (the "v3" default): both HWDGE rings (sync/scalar) carry ONLY
loads so no compute op ever head-of-line blocks a load issue; all compute
runs on DVE; all stores (compute-dependent by nature) go through SWDGE
(gpsimd). 0.5-1 MiB transfers (tile_cols=2048), 6 pool buffers.

Sharding: N=16.7M elements split evenly across 8 NeuronCores; fully
elementwise, no communication.
"""

import numpy as np

import concourse.bass as bass
import concourse.bacc as bacc
import concourse.mybir as mybir
from concourse.tile import TileContext
from concourse.bass_utils import run_bass_kernel_spmd

N = 16777216
N_CORES = 8
SHARD = N // N_CORES  # 2_097_152
P = 128
FREE = SHARD // P  # 16384 elements per partition per core
TILE_COLS = 2048  # 1 MiB f32 / 0.5 MiB bf16 per DMA; 8 tiles per core
N_TILES = FREE // TILE_COLS
F32 = mybir.dt.float32
BF16 = mybir.dt.bfloat16
NP_BF16 = mybir.dt.np(BF16)


def build_program(
    free: int = FREE,
    tile_cols: int = TILE_COLS,
    bufs: int = 6,
    repeats: int = 1,
    hw_loop_repeats: int = 1,
    store_engine: str = "gpsimd",
    load_engine: str = "split",
    x_relu_on_dve: bool = False,
    compute: bool = True,
    schedule: str = "v3",
) -> bass.Bass:
    """hw_loop_repeats wraps the whole body in a tc.For_i hardware loop --
    used only by the timing harness (repeat-differencing)."""
    assert free % tile_cols == 0
    n_tiles = free // tile_cols

    nc = bacc.Bacc(
        "TRN2", target_bir_lowering=False, debug=False, num_devices=N_CORES
    )
    # Each DRAM tile [P, tile_cols] is one fully contiguous block in HBM
    # (best row-buffer locality); the host reshapes to match.
    shape = [n_tiles, P, tile_cols]
    if schedule.startswith("v7"):
        # One input tensor per tile: high | low | x (bf16 riding as f32
        # bit-pairs), one bf16 output tensor: high_out | low_out | x_out.
        # 2 DMAs per tile instead of 6, identical byte counts.
        assert tile_cols % 2 == 0
        in_all = nc.declare_dram_parameter(
            "in_all", [n_tiles, P, 2 * tile_cols + tile_cols // 2], F32,
            isOutput=False,
        )
        out_all = nc.declare_dram_parameter(
            "out_all", [n_tiles, P, 3 * tile_cols], BF16, isOutput=True
        )
        x = x_out = low = high = low_out = high_out = hl = holo = None
    elif schedule.startswith("v6"):
        # high and low packed side by side per tile (cols 0:C / C:2C), and
        # likewise high_out/low_out: halves the DMA count for 4 of the 6
        # streams at identical byte counts.
        x = nc.declare_dram_parameter("x", shape, BF16, isOutput=False)
        x_out = nc.declare_dram_parameter("x_out", shape, BF16, isOutput=True)
        shape2 = [n_tiles, P, 2 * tile_cols]
        hl = nc.declare_dram_parameter("hl", shape2, F32, isOutput=False)
        holo = nc.declare_dram_parameter("holo_out", shape2, BF16, isOutput=True)
        low = high = low_out = high_out = None
    else:
        x = nc.declare_dram_parameter("x", shape, BF16, isOutput=False)
        x_out = nc.declare_dram_parameter("x_out", shape, BF16, isOutput=True)
        low = nc.declare_dram_parameter("low", shape, F32, isOutput=False)
        high = nc.declare_dram_parameter("high", shape, F32, isOutput=False)
        low_out = nc.declare_dram_parameter("low_out", shape, BF16, isOutput=True)
        high_out = nc.declare_dram_parameter("high_out", shape, BF16, isOutput=True)

    relu = mybir.ActivationFunctionType.Relu

    if schedule == "v8":
        # v3 engine roles at tile_cols=4096 with SPLIT tile pools: the
        # load tiles (x,h,l: 40 KB/partition/buf) get a 4-deep pool while
        # the short-lived output tiles (ho,lo: 16 KB) need only 2 -- 192 KB
        # total. Halves the per-op DVE fixed overhead vs C=2048 (20 ops
        # instead of 40 per iteration) while keeping the same bytes of
        # load-ahead slack as bufs=8 at C=2048.
        with TileContext(nc) as tc:
            with tc.tile_pool(name="ld", bufs=bufs) as lpool:
                with tc.tile_pool(name="st", bufs=2) as spool:

                    def body_v8():
                        C = tile_cols
                        for t in range(n_tiles * repeats):
                            ti = t % n_tiles
                            ht = lpool.tile([P, C], F32, tag="h")
                            nc.scalar.dma_start(out=ht[:], in_=high[ti])
                            lt = lpool.tile([P, C], F32, tag="l")
                            nc.sync.dma_start(out=lt[:], in_=low[ti])
                            xt = lpool.tile([P, C], BF16, tag="x")
                            (nc.sync if t % 2 == 0 else nc.scalar).dma_start(
                                out=xt[:], in_=x[ti]
                            )

                            nc.vector.tensor_scalar_max(xt[:], xt[:], 0.0)
                            nc.gpsimd.dma_start(out=x_out[ti], in_=xt[:])

                            ho = spool.tile([P, C], BF16, tag="ho")
                            nc.vector.tensor_scalar_max(ho[:], ht[:], 0.0)
                            nc.gpsimd.dma_start(out=high_out[ti], in_=ho[:])

                            nc.vector.tensor_add(ht[:], lt[:], ht[:])
                            nc.vector.tensor_scalar(
                                ht[:], ht[:], 0.0, None, mybir.AluOpType.is_ge
                            )
                            lo = spool.tile([P, C], BF16, tag="lo")
                            nc.vector.tensor_mul(lo[:], ht[:], lt[:])
                            nc.gpsimd.dma_start(out=low_out[ti], in_=lo[:])

                    if hw_loop_repeats > 1:
                        with tc.For_i(0, hw_loop_repeats, 1):
                            body_v8()
                    else:
                        body_v8()
        nc.compile()
        return nc

    with TileContext(nc) as tc:
        with tc.tile_pool(name="io", bufs=bufs) as pool:
            engines = {"scalar": nc.scalar, "gpsimd": nc.gpsimd, "sync": nc.sync}

            def eng_for(stream: str, t: int):
                """Resolve the DMA-issuing engine for stream in
                {x,h,l,xo,ho,lo} at tile t. Loads stay on the two HWDGE
                rings (sync/scalar) so they are never head-of-line blocked
                behind stores, which wait on compute; stores go to SWDGE
                (gpsimd) by default."""
                if stream in ("x", "h", "l"):
                    if load_engine == "split":
                        # balance HWDGE ring bytes: h(f32) on scalar,
                        # l(f32) on sync, x(bf16, half-size) alternates
                        if stream == "h":
                            return engines["scalar"]
                        if stream == "l":
                            return engines["sync"]
                        return engines["sync" if t % 2 == 0 else "scalar"]
                    return engines[load_engine]
                if store_engine == "mix":
                    return engines["scalar" if stream == "xo" else "gpsimd"]
                if store_engine == "alt":
                    return engines["gpsimd" if t % 2 == 0 else "scalar"]
                return engines[store_engine]

            def body():
                for t in range(n_tiles * repeats):
                    ti = t % n_tiles

                    xt = pool.tile([P, tile_cols], BF16, tag="x")
                    eng_for("x", t).dma_start(out=xt[:], in_=x[ti])
                    if compute:
                        if x_relu_on_dve:
                            nc.vector.tensor_scalar_max(xt[:], xt[:], 0.0)
                        else:
                            nc.scalar.activation(xt[:], xt[:], relu)
                    eng_for("xo", t).dma_start(out=x_out[ti], in_=xt[:])

                    ht = pool.tile([P, tile_cols], F32, tag="h")
                    eng_for("h", t).dma_start(out=ht[:], in_=high[ti])
                    lt = pool.tile([P, tile_cols], F32, tag="l")
                    eng_for("l", t).dma_start(out=lt[:], in_=low[ti])

                    if not compute:
                        # DMA-floor diagnostic: identical transfer shapes,
                        # no compute ops (stores the loaded bytes as-is)
                        eng_for("ho", t).dma_start(
                            out=high_out[ti],
                            in_=ht[:].bitcast(BF16)[:, 0:tile_cols],
                        )
                        eng_for("lo", t).dma_start(
                            out=low_out[ti],
                            in_=lt[:].bitcast(BF16)[:, 0:tile_cols],
                        )
                        continue

                    ho = pool.tile([P, tile_cols], BF16, tag="ho")
                    nc.scalar.activation(ho[:], ht[:], relu)  # f32 -> bf16
                    eng_for("ho", t).dma_start(out=high_out[ti], in_=ho[:])

                    # s = low + high computed in place over ht (f32, exact);
                    # mask = (s >= 0); low_out = mask * low, rounded to bf16
                    nc.vector.tensor_add(ht[:], lt[:], ht[:])
                    nc.vector.tensor_scalar(
                        ht[:], ht[:], 0.0, None, mybir.AluOpType.is_ge
                    )
                    lo = pool.tile([P, tile_cols], BF16, tag="lo")
                    nc.vector.tensor_mul(lo[:], ht[:], lt[:])
                    eng_for("lo", t).dma_start(out=low_out[ti], in_=lo[:])

            def body_v3():
                """Both HWDGE rings are pure load streams; all compute on
                DVE; all stores on SWDGE."""
                for t in range(n_tiles * repeats):
                    ti = t % n_tiles

                    ht = pool.tile([P, tile_cols], F32, tag="h")
                    nc.scalar.dma_start(out=ht[:], in_=high[ti])
                    lt = pool.tile([P, tile_cols], F32, tag="l")
                    nc.sync.dma_start(out=lt[:], in_=low[ti])
                    xt = pool.tile([P, tile_cols], BF16, tag="x")
                    (nc.sync if t % 2 == 0 else nc.scalar).dma_start(
                        out=xt[:], in_=x[ti]
                    )

                    nc.vector.tensor_scalar_max(xt[:], xt[:], 0.0)
                    nc.gpsimd.dma_start(out=x_out[ti], in_=xt[:])

                    ho = pool.tile([P, tile_cols], BF16, tag="ho")
                    nc.vector.tensor_scalar_max(ho[:], ht[:], 0.0)  # f32->bf16
                    nc.gpsimd.dma_start(out=high_out[ti], in_=ho[:])

                    nc.vector.tensor_add(ht[:], lt[:], ht[:])
                    nc.vector.tensor_scalar(
                        ht[:], ht[:], 0.0, None, mybir.AluOpType.is_ge
                    )
                    lo = pool.tile([P, tile_cols], BF16, tag="lo")
                    nc.vector.tensor_mul(lo[:], ht[:], lt[:])
                    nc.gpsimd.dma_start(out=low_out[ti], in_=lo[:])

            def body_v4(prefetch: int = 2, ho_on_dve: bool = False):
                """Software-prefetched loads: tile t+PF's loads are issued
                before tile t's compute in every engine's program order, so
                a compute op on scalar never delays a load issue by more
                than the PF-tile slack. Loads: h->scalar, l->sync, x
                alternating; relus on scalar (DVE keeps only the 3-op low
                chain); stores on gpsimd."""
                total = n_tiles * repeats

                def issue_loads(t):
                    ti = t % n_tiles
                    ht = pool.tile([P, tile_cols], F32, tag="h")
                    nc.scalar.dma_start(out=ht[:], in_=high[ti])
                    lt = pool.tile([P, tile_cols], F32, tag="l")
                    nc.sync.dma_start(out=lt[:], in_=low[ti])
                    xt = pool.tile([P, tile_cols], BF16, tag="x")
                    (nc.sync if t % 2 == 0 else nc.scalar).dma_start(
                        out=xt[:], in_=x[ti]
                    )
                    return xt, ht, lt

                from collections import deque

                q = deque()
                for t in range(min(prefetch, total)):
                    q.append(issue_loads(t))
                for t in range(total):
                    if t + prefetch < total:
                        q.append(issue_loads(t + prefetch))
                    ti = t % n_tiles
                    xt, ht, lt = q.popleft()

                    nc.scalar.activation(xt[:], xt[:], relu)
                    nc.gpsimd.dma_start(out=x_out[ti], in_=xt[:])

                    ho = pool.tile([P, tile_cols], BF16, tag="ho")
                    if ho_on_dve == "alt":
                        if t % 2 == 0:
                            nc.vector.tensor_scalar_max(ho[:], ht[:], 0.0)
                        else:
                            nc.scalar.activation(ho[:], ht[:], relu)
                    elif ho_on_dve:
                        nc.vector.tensor_scalar_max(ho[:], ht[:], 0.0)
                    else:
                        nc.scalar.activation(ho[:], ht[:], relu)
                    nc.gpsimd.dma_start(out=high_out[ti], in_=ho[:])

                    nc.vector.tensor_add(ht[:], lt[:], ht[:])
                    nc.vector.tensor_scalar(
                        ht[:], ht[:], 0.0, None, mybir.AluOpType.is_ge
                    )
                    lo = pool.tile([P, tile_cols], BF16, tag="lo")
                    nc.vector.tensor_mul(lo[:], ht[:], lt[:])
                    nc.gpsimd.dma_start(out=low_out[ti], in_=lo[:])

            def body_v6(x_on_scalar: bool = False):
                """v3 engine roles with fused streams: one 2C-wide f32 load
                carries high|low, one 2C-wide bf16 store carries
                high_out|low_out; compute addresses the halves as slices."""
                C = tile_cols
                for t in range(n_tiles * repeats):
                    ti = t % n_tiles

                    hlt = pool.tile([P, 2 * C], F32, tag="hl")
                    (nc.sync if t % 2 == 0 else nc.scalar).dma_start(
                        out=hlt[:], in_=hl[ti]
                    )
                    xt = pool.tile([P, C], BF16, tag="x")
                    (nc.scalar if t % 2 == 0 else nc.sync).dma_start(
                        out=xt[:], in_=x[ti]
                    )

                    if x_on_scalar:
                        nc.scalar.activation(xt[:], xt[:], relu)
                    else:
                        nc.vector.tensor_scalar_max(xt[:], xt[:], 0.0)
                    nc.gpsimd.dma_start(out=x_out[ti], in_=xt[:])

                    ht = hlt[:, 0:C]
                    lt = hlt[:, C : 2 * C]
                    holot = pool.tile([P, 2 * C], BF16, tag="holo")
                    hot = holot[:, 0:C]
                    lot = holot[:, C : 2 * C]

                    nc.vector.tensor_scalar_max(hot, ht, 0.0)  # f32->bf16
                    nc.vector.tensor_add(ht, lt, ht)
                    nc.vector.tensor_scalar(
                        ht, ht, 0.0, None, mybir.AluOpType.is_ge
                    )
                    nc.vector.tensor_mul(lot, ht, lt)
                    nc.gpsimd.dma_start(out=holo[ti], in_=holot[:])

            def body_v9():
                """Perfect engine-role separation: sync ring carries ALL
                loads (one deep pure-load queue), scalar is pure compute
                (both relus, zero DMA issues), DVE keeps only the 3-op
                mask chain, stores on SWDGE."""
                for t in range(n_tiles * repeats):
                    ti = t % n_tiles

                    ht = pool.tile([P, tile_cols], F32, tag="h")
                    nc.sync.dma_start(out=ht[:], in_=high[ti])
                    lt = pool.tile([P, tile_cols], F32, tag="l")
                    nc.sync.dma_start(out=lt[:], in_=low[ti])
                    xt = pool.tile([P, tile_cols], BF16, tag="x")
                    nc.sync.dma_start(out=xt[:], in_=x[ti])

                    nc.scalar.activation(xt[:], xt[:], relu)
                    nc.gpsimd.dma_start(out=x_out[ti], in_=xt[:])

                    ho = pool.tile([P, tile_cols], BF16, tag="ho")
                    nc.scalar.activation(ho[:], ht[:], relu)  # f32->bf16
                    nc.gpsimd.dma_start(out=high_out[ti], in_=ho[:])

                    nc.vector.tensor_add(ht[:], lt[:], ht[:])
                    nc.vector.tensor_scalar(
                        ht[:], ht[:], 0.0, None, mybir.AluOpType.is_ge
                    )
                    lo = pool.tile([P, tile_cols], BF16, tag="lo")
                    nc.vector.tensor_mul(lo[:], ht[:], lt[:])
                    nc.gpsimd.dma_start(out=low_out[ti], in_=lo[:])

            def body_v7():
                """Maximal fusion: ONE f32 load (high|low|x-bits) and ONE
                bf16 store (high_out|low_out|x_out) per tile; loads
                alternate HWDGE rings, stores on SWDGE, compute on DVE."""
                C = tile_cols
                for t in range(n_tiles * repeats):
                    ti = t % n_tiles

                    it = pool.tile([P, 2 * C + C // 2], F32, tag="in")
                    (nc.sync if t % 2 == 0 else nc.scalar).dma_start(
                        out=it[:], in_=in_all[ti]
                    )
                    ot = pool.tile([P, 3 * C], BF16, tag="out")

                    ht = it[:, 0:C]
                    lt = it[:, C : 2 * C]
                    xt = it[:, 2 * C : 2 * C + C // 2].bitcast(BF16)

                    nc.vector.tensor_scalar_max(ot[:, 2 * C : 3 * C], xt, 0.0)
                    nc.vector.tensor_scalar_max(ot[:, 0:C], ht, 0.0)
                    nc.vector.tensor_add(ht, lt, ht)
                    nc.vector.tensor_scalar(
                        ht, ht, 0.0, None, mybir.AluOpType.is_ge
                    )
                    nc.vector.tensor_mul(ot[:, C : 2 * C], ht, lt)
                    nc.gpsimd.dma_start(out=out_all[ti], in_=ot[:])

            # Note: a DRAM->DRAM max-accum DMA ("relu in the SDMA CCE",
            # bypassing SBUF for the x stream) was tried and is rejected by
            # the compiler (assertDMACopySupportedCceOp).
            body_fn = {
                "v1": body,
                "v3": body_v3,
                "v4": body_v4,
                "v4d": lambda: body_v4(ho_on_dve=True),
                "v4e": lambda: body_v4(ho_on_dve="alt"),
                "v4dp3": lambda: body_v4(prefetch=3, ho_on_dve=True),
                "v4p3": lambda: body_v4(prefetch=3),
                "v4p4": lambda: body_v4(prefetch=4),
                "v6": body_v6,
                "v6s": lambda: body_v6(x_on_scalar=True),
                "v7": body_v7,
                "v9": body_v9,
            }[schedule]
            if hw_loop_repeats > 1:
                with tc.For_i(0, hw_loop_repeats, 1):
                    body_fn()
            else:
                body_fn()
    nc.compile()
    return nc


def pack_full(x, low, high, tile_cols: int = TILE_COLS, schedule: str = "v3"):
    """Full f32 arrays -> dict of full arrays in the declared dtypes/layout
    (x cast to bf16; contiguous-tile layout [N_CORES*n_tiles, P, C]; fused
    schedules pack high|low side by side per tile)."""
    x = np.ascontiguousarray(np.asarray(x, dtype=np.float32).reshape(-1))
    low = np.ascontiguousarray(np.asarray(low, dtype=np.float32).reshape(-1))
    high = np.ascontiguousarray(np.asarray(high, dtype=np.float32).reshape(-1))
    assert x.shape == (N,), x.shape
    nt = N // (P * tile_cols)
    xb = x.astype(NP_BF16).reshape(nt, P, tile_cols)  # round-to-nearest-even
    if schedule.startswith("v7"):
        in_all = np.concatenate(
            [
                high.reshape(nt, P, tile_cols),
                low.reshape(nt, P, tile_cols),
                np.ascontiguousarray(xb).view(np.float32),  # bf16 bit-pairs
            ],
            axis=2,
        )
        return {"in_all": in_all}
    if schedule.startswith("v6"):
        hl = np.concatenate(
            [high.reshape(nt, P, tile_cols), low.reshape(nt, P, tile_cols)],
            axis=2,
        )
        return {"x": xb, "hl": hl}
    return {
        "x": xb,
        "low": low.reshape(nt, P, tile_cols),
        "high": high.reshape(nt, P, tile_cols),
    }


def unpack_outputs(by_name, schedule: str = "v3"):
    """name -> full bf16 array, back to (x_out, low_out, high_out) f32."""
    if schedule.startswith("v7"):
        oa = np.asarray(by_name["out_all"])
        C = oa.shape[-1] // 3
        return (
            oa[..., 2 * C : 3 * C].reshape(-1).astype(np.float32),
            oa[..., C : 2 * C].reshape(-1).astype(np.float32),
            oa[..., 0:C].reshape(-1).astype(np.float32),
        )
    x_out = np.asarray(by_name["x_out"]).reshape(-1).astype(np.float32)
    if schedule.startswith("v6"):
        holo = np.asarray(by_name["holo_out"])
        C = holo.shape[-1] // 2
        high_out = holo[..., 0:C].reshape(-1).astype(np.float32)
        low_out = holo[..., C : 2 * C].reshape(-1).astype(np.float32)
    else:
        low_out = np.asarray(by_name["low_out"]).reshape(-1).astype(np.float32)
        high_out = np.asarray(by_name["high_out"]).reshape(-1).astype(np.float32)
    return x_out, low_out, high_out


def shard_inputs(x, low, high, tile_cols: int = TILE_COLS, schedule: str = "v3"):
    """Per-core input dicts (spmd fallback / timing harness)."""
    full = pack_full(x, low, high, tile_cols, schedule)
    nt_core = FREE // tile_cols
    in_maps = []
    for c in range(N_CORES):
        s = slice(c * nt_core, (c + 1) * nt_core)
        in_maps.append({nm: a[s] for nm, a in full.items()})
    return in_maps


_NC = None

DEFAULT_SCHEDULE = "v3"

# Build configuration used by kernel(); test.py may override before the
# first kernel() call to validate a specific variant end to end.
DEFAULT_BUILD: dict = {}


def _get_nc() -> bass.Bass:
    global _NC
    if _NC is None:
        _NC = build_program(**DEFAULT_BUILD)
    return _NC


_RUNNER = None


def _make_runner(nc):
    """Cached PJRT runner (mirrors bass2jax.run_bass_via_pjrt, but the jitted
    callable is built once so repeat kernel() calls skip re-tracing). No
    donation: this kernel writes every output element, so the zero 'output'
    operands are reusable dummies and XLA result buffers may start uninit."""
    import jax
    from jax.sharding import Mesh, PartitionSpec, NamedSharding
    from jax.experimental.shard_map import shard_map
    from concourse.bass2jax import (
        _bass_exec_p,
        install_neuronx_cc_hook,
        partition_id_tensor,
    )

    install_neuronx_cc_hook()
    partition_name = nc.partition_id_tensor.name if nc.partition_id_tensor else None

    in_names, out_names, out_avals, zero_shapes = [], [], [], []
    in_shapes = {}
    for alloc in nc.m.functions[0].allocations:
        if not isinstance(alloc, mybir.MemoryLocationSet):
            continue
        name = alloc.memorylocations[0].name
        if alloc.kind == "ExternalInput":
            if name != partition_name:
                in_names.append(name)
                in_shapes[name] = tuple(alloc.tensor_shape)
        elif alloc.kind == "ExternalOutput":
            shape = tuple(alloc.tensor_shape)
            dtype = mybir.dt.np(alloc.dtype)
            out_names.append(name)
            out_avals.append(jax.core.ShapedArray(shape, dtype))
            zero_shapes.append((shape, dtype))
    n_params = len(in_names)
    all_in_names = list(in_names) + list(out_names)
    if partition_name is not None:
        all_in_names.append(partition_name)

    def _body(*args):
        operands = list(args)
        if partition_name is not None:
            operands.append(partition_id_tensor())
        outs = _bass_exec_p.bind(
            *operands,
            out_avals=tuple(out_avals),
            in_names=tuple(all_in_names),
            out_names=tuple(out_names),
            lowering_input_output_aliases=(),
            sim_require_finite=True,
            sim_require_nnan=True,
            nc=nc,
        )
        return tuple(outs)

    devices = jax.devices()[:N_CORES]
    mesh = Mesh(np.asarray(devices), ("core",))
    n_io = n_params + len(out_names)
    sharded = jax.jit(
        shard_map(
            _body,
            mesh=mesh,
            in_specs=(PartitionSpec("core"),) * n_io,
            out_specs=(PartitionSpec("core"),) * len(out_names),
            check_rep=False,
        ),
        keep_unused=True,
    )
    sharding = NamedSharding(mesh, PartitionSpec("core"))
    zeros = [
        jax.device_put(np.zeros((N_CORES * s[0], *s[1:]), d), sharding)
        for (s, d) in zero_shapes
    ]

    def run(full_by_name):
        """full_by_name: name -> full flat array already in the declared
        dtype. Sharding across cores is just the row-major split of axis 0
        after reshaping to (N_CORES*n_tiles, P, tile_cols), so device_put
        with the core sharding does the whole distribution."""
        dev_in = []
        for nm in in_names:
            s = in_shapes[nm]
            a = full_by_name[nm].reshape((N_CORES * s[0],) + tuple(s[1:]))
            dev_in.append(jax.device_put(a, sharding))
        outs = sharded(*dev_in, *zeros)
        return {nm: np.asarray(outs[i]) for i, nm in enumerate(out_names)}

    return run


def kernel(x: np.ndarray, low: np.ndarray, high: np.ndarray, **_run_kwargs):
    nc = _get_nc()
    tile_cols = DEFAULT_BUILD.get("tile_cols", TILE_COLS)
    schedule = DEFAULT_BUILD.get("schedule", DEFAULT_SCHEDULE)
    global _RUNNER
    results = None
    if not _run_kwargs:
        # Fast path: cached jitted executable (no per-call re-trace), fed
        # with full arrays (device_put's row-major axis-0 split IS the
        # per-core sharding).
        try:
            if _RUNNER is None:
                _RUNNER = _make_runner(nc)
            by_name = _RUNNER(pack_full(x, low, high, tile_cols, schedule))
            return unpack_outputs(by_name, schedule)
        except Exception:
            _RUNNER = None
            results = None

    if results is None:
        in_maps = shard_inputs(x, low, high, tile_cols, schedule)
        res = None
        for attempt in range(3):
            try:
                res = run_bass_kernel_spmd(
                    nc, in_maps, list(range(N_CORES)), **_run_kwargs
                )
                break
            except Exception:
                # Transient device wedge (NRT_EXEC_UNIT_UNRECOVERABLE) -- reset
                # the jax backend so the next attempt re-establishes the mesh.
                if attempt == 2:
                    raise
                import time as _time

                try:
                    import jax

                    jax.clear_caches()
                    jax.extend.backend.clear_backends()
                except Exception:
                    pass
                _time.sleep(10.0)
        results = res.results
        if _run_kwargs:
            kernel.last_results = res  # expose trace/profile to test harness

    by_name = {
        nm: np.concatenate([np.asarray(results[c][nm]) for c in range(N_CORES)])
        for nm in results[0]
    }
    return unpack_outputs(by_name, schedule)
